# revision 2
# baseline (speedup 1.0000x reference)
"""LSTMCell (B=65536, H=512) Bass/Tile kernel for 8 trn2 NeuronCores.

Data-parallel over batch: each core processes 8192 rows.
Per 128-row tile:
  z = x + stm                     (DVE)
  zT chunks via PE transpose      (TensorE identity matmul -> PSUM)
  gates = zT.T @ W_packed (f32r)  (TensorE, accumulate over 4 k-chunks)
  pre = gates + bias              (DVE, one [128,2048] op)
  sigmoid(f,i,o) / tanh(g)        (ACT, two ops)
  c = sf + si*tg; h = tanh(c)*so  (DVE + ACT)
"""

import os
import sys

if "/opt/trn_rl_repo" not in sys.path:
    sys.path.insert(0, "/opt/trn_rl_repo")

import numpy as np

import concourse.bacc as bacc
import concourse.mybir as mybir
import concourse.tile as tile

N_CORES = 8
B, H = 65536, 512
B_CORE = B // N_CORES  # 8192
F32 = mybir.dt.float32
F32R = mybir.dt.float32r
AF = mybir.ActivationFunctionType

NEFF_DUMP = "/tmp/lstm_kernel.neff"

# gate order in the packed weight/bias/psum layout: sigmoid gates first so one
# ACT op covers [0:1536], tanh gate last at [1536:2048]
#   slot 0: f (sigmoid), 1: i (sigmoid), 2: o (sigmoid), 3: g (tanh)


def build_module(b_core=B_CORE, n_cores=N_CORES):
    nc = bacc.Bacc(
        "TRN2",
        target_bir_lowering=False,
        debug=False,
        num_devices=n_cores,
    )
    x = nc.dram_tensor("x", [b_core, H], F32, kind="ExternalInput").ap()
    s = nc.dram_tensor("s", [b_core, H], F32, kind="ExternalInput").ap()
    wt = nc.dram_tensor("wt", [128, 8192], F32R, kind="ExternalInput").ap()
    bias = nc.dram_tensor("bias", [128, 2048], F32, kind="ExternalInput").ap()
    ident = nc.dram_tensor("ident", [128, 128], F32, kind="ExternalInput").ap()
    out = nc.dram_tensor("out", [2, b_core, H], F32, kind="ExternalOutput").ap()

    n_tiles = b_core // 128

    with tile.TileContext(nc) as tc:
        with (
            tc.tile_pool(name="const", bufs=1) as cpool,
            tc.tile_pool(name="work", bufs=3) as pool,
            tc.tile_pool(name="pzt", bufs=2, space="PSUM") as pzt,
            tc.tile_pool(name="pgates", bufs=1, space="PSUM") as pg,
        ):
            wt_sb = cpool.tile([128, 8192], F32R)
            nc.sync.dma_start(out=wt_sb[:], in_=wt[:])
            bias_sb = cpool.tile([128, 2048], F32)
            nc.sync.dma_start(out=bias_sb[:], in_=bias[:])
            id_sb = cpool.tile([128, 128], F32)
            nc.sync.dma_start(out=id_sb[:], in_=ident[:])

            for t in range(n_tiles):
                rows = slice(t * 128, (t + 1) * 128)
                x_t = pool.tile([128, H], F32, tag="x")
                nc.sync.dma_start(out=x_t[:], in_=x[rows, :])
                s_t = pool.tile([128, H], F32, tag="s")
                nc.sync.dma_start(out=s_t[:], in_=s[rows, :])
                z_t = pool.tile([128, H], F32, tag="z")
                nc.vector.tensor_add(z_t[:], x_t[:], s_t[:])

                # transpose z into [h_local, b] chunks (PSUM), evacuate to SBUF
                zt_ps = pzt.tile([128, H], F32, tag="ztp")
                for k in range(4):
                    nc.tensor.transpose(
                        zt_ps[:, k * 128 : (k + 1) * 128],
                        z_t[:, k * 128 : (k + 1) * 128],
                        id_sb[:],
                    )
                zt_sb = pool.tile([128, H], F32R, tag="zt")
                nc.vector.tensor_copy(zt_sb[:], zt_ps[:])

                # gates[b, slot*512+j] = sum_h z[b,h] * W_slot[j,h]
                g_ps = pg.tile([128, 2048], F32, tag="gates")
                for k in range(4):
                    lhs = zt_sb[:, k * 128 : (k + 1) * 128]
                    for gs in range(4):
                        nc.tensor.matmul(
                            g_ps[:, gs * 512 : (gs + 1) * 512],
                            lhs,
                            wt_sb[
                                :, gs * 2048 + k * 512 : gs * 2048 + (k + 1) * 512
                            ],
                            start=(k == 0),
                            stop=(k == 3),
                        )

                pre = pool.tile([128, 2048], F32, tag="pre")
                nc.vector.tensor_add(pre[:], g_ps[:], bias_sb[:])
                acts = pool.tile([128, 2048], F32, tag="acts")
                nc.scalar.activation(acts[:, 0:1536], pre[:, 0:1536], AF.Sigmoid)
                nc.scalar.activation(acts[:, 1536:2048], pre[:, 1536:2048], AF.Tanh)

                prod = pool.tile([128, H], F32, tag="prod")
                nc.vector.tensor_mul(prod[:], acts[:, 512:1024], acts[:, 1536:2048])
                c_t = pool.tile([128, H], F32, tag="c")
                nc.vector.tensor_add(c_t[:], acts[:, 0:512], prod[:])
                tc_t = pool.tile([128, H], F32, tag="tc")
                nc.scalar.activation(tc_t[:], c_t[:], AF.Tanh)
                h_t = pool.tile([128, H], F32, tag="h")
                nc.vector.tensor_mul(h_t[:], tc_t[:], acts[:, 1024:1536])

                nc.sync.dma_start(out=out[0, rows, :], in_=c_t[:])
                nc.sync.dma_start(out=out[1, rows, :], in_=h_t[:])

    nc.compile()
    return nc


def round_fp32r(a):
    """Round fp32 to the fp32r grid (11-bit mantissa, RNE)."""
    u = np.ascontiguousarray(a, np.float32).view(np.uint32)
    r = (u.astype(np.uint64) + 0x7FF + ((u >> 12) & 1)) & 0xFFFFF000
    return r.astype(np.uint32).view(np.float32)


def pack_inputs(inputs, short_term_memory, Wf, bf, Wi, bi, Wg, bg, Wo, bo):
    x = np.ascontiguousarray(np.asarray(inputs, np.float32))
    s = np.ascontiguousarray(np.asarray(short_term_memory, np.float32))
    Ws = [Wf, Wi, Wo, Wg]
    bs = [bf, bi, bo, bg]
    wt = np.empty((128, 8192), np.float32)
    for gs, W in enumerate(Ws):
        Wt = np.ascontiguousarray(np.asarray(W, np.float32).T)  # [h, j] = W[j, h]
        # wt[p, gs*2048 + k*512 + j] = W[j, k*128+p]
        wt[:, gs * 2048 : (gs + 1) * 2048] = round_fp32r(
            Wt.reshape(4, 128, 512).transpose(1, 0, 2).reshape(128, 2048)
        )
    bias = np.empty((128, 2048), np.float32)
    for gs, b in enumerate(bs):
        bias[:, gs * 512 : (gs + 1) * 512] = np.asarray(b, np.float32)[None, :]
    ident = np.eye(128, dtype=np.float32)
    return {"x": x, "s": s, "wt": wt, "bias": bias, "ident": ident}


class Runner:
    """Compiles the module once and keeps a reusable jitted executor."""

    def __init__(self, nc=None, n_cores=N_CORES):
        import jax
        from concourse import bass2jax as b2j

        self.jax = jax
        self.n_cores = n_cores
        self.nc = nc or build_module(n_cores=n_cores)
        b2j.install_neuronx_cc_hook()

        # dump the final (renamed) NEFF so neuron-profile can pair it with NTFFs
        if not getattr(b2j, "_neff_dump_patched", False):
            orig = b2j.rename_neff_tensors_and_patch_header

            def _patched(neff_path, mapping):
                data = orig(neff_path, mapping)
                with open(NEFF_DUMP, "wb") as f:
                    f.write(data)
                return data

            b2j.rename_neff_tensors_and_patch_header = _patched
            b2j._neff_dump_patched = True

        from jax.experimental.shard_map import shard_map
        from jax.sharding import Mesh, NamedSharding, PartitionSpec

        part_name = (
            self.nc.partition_id_tensor.name if self.nc.partition_id_tensor else None
        )
        in_names, out_names, out_avals = [], [], []
        self.out_shapes = {}
        for alloc in self.nc.m.functions[0].allocations:
            if not isinstance(alloc, mybir.MemoryLocationSet):
                continue
            name = alloc.memorylocations[0].name
            if alloc.kind == "ExternalInput":
                if name != part_name:
                    in_names.append(name)
            elif alloc.kind == "ExternalOutput":
                out_names.append(name)
                shape = tuple(alloc.tensor_shape)
                dt = mybir.dt.np(alloc.dtype)
                out_avals.append(jax.core.ShapedArray(shape, dt))
                self.out_shapes[name] = (shape, dt)
        self.in_names, self.out_names = in_names, out_names
        nc_ref = self.nc

        bind_names = list(in_names) + list(out_names)
        if part_name is not None:
            bind_names.append(part_name)

        def _body(*args):
            operands = list(args)
            if part_name is not None:
                operands.append(b2j.partition_id_tensor())
            outs = b2j._bass_exec_p.bind(
                *operands,
                out_avals=tuple(out_avals),
                in_names=tuple(bind_names),
                out_names=tuple(out_names),
                lowering_input_output_aliases=(),
                sim_require_finite=False,
                sim_require_nnan=False,
                nc=nc_ref,
            )
            return tuple(outs)

        devices = jax.devices()[: self.n_cores]
        mesh = Mesh(np.asarray(devices), ("core",))
        spec = PartitionSpec("core")
        n_args = len(in_names) + len(out_names)
        self.sharding = NamedSharding(mesh, spec)
        self.fn = jax.jit(
            shard_map(
                _body,
                mesh=mesh,
                in_specs=(spec,) * n_args,
                out_specs=(spec,) * len(out_names),
                check_rep=False,
            ),
            keep_unused=True,
        )
        self._dev_args = None

    def stage(self, packed):
        """Transfer inputs (sharded/replicated as needed) to devices once."""
        jax = self.jax
        nc_n = self.n_cores
        args = []
        for name in self.in_names:
            a = packed[name]
            if name in ("x", "s"):
                glob = a  # already [B, H]; shard axis 0 into 8
            else:
                glob = np.concatenate([a] * nc_n, axis=0)  # replicate
            args.append(glob)
        for name in self.out_names:
            shape, dt = self.out_shapes[name]
            args.append(np.zeros((shape[0] * nc_n,) + shape[1:], dt))
        self._host_args = args
        self._dev_args = [jax.device_put(a, self.sharding) for a in args]

    def execute(self):
        outs = self.fn(*self._dev_args)
        self.jax.block_until_ready(outs)
        return outs

    def run(self, packed):
        self.stage(packed)
        outs = self.execute()
        res = {}
        for name, arr in zip(self.out_names, outs):
            a = np.asarray(arr)  # [n_cores*d0, ...]
            shape, _ = self.out_shapes[name]
            res[name] = a.reshape((self.n_cores, shape[0]) + tuple(shape[1:]))
        return res


_RUNNER = None


def _get_runner():
    global _RUNNER
    if _RUNNER is None:
        _RUNNER = Runner()
    return _RUNNER


def kernel(**inputs):
    r = _get_runner()
    packed = pack_inputs(**inputs)
    res = r.run(packed)
    per_core = res["out"]  # [8, 2, 8192, 512]
    return np.ascontiguousarray(
        per_core.transpose(1, 0, 2, 3).reshape(2, B, H)
    )


if __name__ == "__main__":
    nc = build_module()
    print("module built + compiled OK")



# revision 3
# speedup vs baseline: 2.2458x; 2.2458x over previous
"""LSTMCell (B=65536, H=512) Bass/Tile kernel for 8 trn2 NeuronCores — v2.

Data-parallel over batch (8192 rows/core), fully transposed layout so no
on-device transposes are needed:
  host packs xT,sT as [h, b] bf16 chunks; device computes zT = xT + sT (DVE)
  gates[j, b] = sum_h W[j,h] * zT[h,b]  (TensorE, W chunks stationary, bf16,
                                         N=512 per matmul, fp32 PSUM)
  act = sigmoid/tanh(gates + bias[j])   (ScalarE reads PSUM, per-partition
                                         bias fused into the activation op)
  c = sf + si*tg; h = tanh(c)*so        (DVE bf16 + ScalarE)
  outputs stay transposed [j, b] bf16; host transposes back to [b, j] f32.

Per super-tile of 1024 batch rows: one 2MB DMA in, 128 matmuls, 16 fused
bias+activation ops, 12 DVE ops, one 2MB DMA out.
"""

import os
import sys

if "/opt/trn_rl_repo" not in sys.path:
    sys.path.insert(0, "/opt/trn_rl_repo")

import numpy as np

import concourse.bacc as bacc
import concourse.mybir as mybir
import concourse.tile as tile

N_CORES = 8
B, H = 65536, 512
B_CORE = B // N_CORES  # 8192
SUPER = 1024  # batch rows per super-tile
N_SUPER = B_CORE // SUPER  # 8
F32 = mybir.dt.float32
BF16 = mybir.dt.bfloat16
BF16NP = mybir.dt.np(mybir.dt.bfloat16)
AF = mybir.ActivationFunctionType

NEFF_DUMP = "/tmp/lstm_kernel.neff"

# gate order: 0=f (sigmoid), 1=i (sigmoid), 2=o (sigmoid), 3=g (tanh)
# g16 = jc*4 + gate indexes the 16 [128j] output chunks (chunk-major, so all
# four gates of j-chunk jc complete by g16 = jc*4+3 and the epilogue for that
# chunk can start while later chunks are still in the matmul).


def build_module(b_core=B_CORE, n_cores=N_CORES):
    nc = bacc.Bacc(
        "TRN2",
        target_bir_lowering=False,
        debug=False,
        num_devices=n_cores,
    )
    # xs[p, k, 0, b] = x[b, k*128+p]; xs[p, k, 1, b] = s[b, k*128+p]
    xs = nc.dram_tensor("xs", [128, 4, 2, b_core], BF16, kind="ExternalInput").ap()
    # wt[p, ((jc*4+gate)*4 + k)*128 + j] = W_gate[jc*128 + j, k*128 + p]
    wt = nc.dram_tensor("wt", [128, 8192], BF16, kind="ExternalInput").ap()
    # bias[p, jc*4+gate] = b_gate[jc*128 + p]
    bias = nc.dram_tensor("bias", [128, 16], F32, kind="ExternalInput").ap()
    # out[p, jc, 0, b] = c[b, jc*128+p]; out[p, jc, 1, b] = h[b, jc*128+p]
    out = nc.dram_tensor("out", [128, 4, 2, b_core], BF16, kind="ExternalOutput").ap()

    with tile.TileContext(nc) as tc:
        with (
            tc.tile_pool(name="const", bufs=1) as cpool,
            tc.tile_pool(name="io", bufs=2) as iopool,
            tc.tile_pool(name="zed", bufs=2) as zpool,
            tc.tile_pool(name="acts", bufs=2) as apool,
            tc.tile_pool(name="epi", bufs=2) as epool,
            tc.tile_pool(name="outp", bufs=2) as opool,
            tc.tile_pool(name="ps", bufs=3, space="PSUM") as pspool,
        ):
            # first super's k0 input slice goes first in the queue so compute
            # can start as early as possible; weights follow in 4 chunks so
            # j-chunk 0's matmuls only wait on weight chunk 0
            def load_xs(pool, b0):
                # xs pieces per k-chunk: piece k holds (x_k, s_k) so z_k can
                # be computed as soon as that 512KB lands
                t = pool.tile([128, 4, 2, SUPER], BF16, tag="xs")
                for k in range(4):
                    nc.sync.dma_start(
                        out=t[:, k, :, :], in_=xs[:, k, :, b0 : b0 + SUPER]
                    )
                return t

            xs_first = load_xs(iopool, 0)
            bias_sb = cpool.tile([128, 16], F32)
            nc.sync.dma_start(out=bias_sb[:], in_=bias[:])
            wt_sbs = []
            for c in range(4):
                w_part = cpool.tile([128, 2048], BF16, name=f"wt_sb{c}")
                nc.sync.dma_start(out=w_part[:], in_=wt[:, c * 2048 : (c + 1) * 2048])
                wt_sbs.append(w_part)

            for sp in range(N_SUPER):
                b0 = sp * SUPER
                xs_t = xs_first if sp == 0 else load_xs(iopool, b0)
                z_t = zpool.tile([128, 4, SUPER], BF16, tag="z")
                for k in range(4):
                    nc.vector.tensor_add(
                        z_t[:, k, :], xs_t[:, k, 0, :], xs_t[:, k, 1, :]
                    )

                acts_t = apool.tile([128, 16, SUPER], BF16, tag="acts")
                for g16 in range(16):
                    ps = pspool.tile([128, SUPER], F32, tag="ps")
                    # k outer / half inner: each weight chunk is stationary
                    # for both 512-col halves (one LDWEIGHTS serves 2 matmuls)
                    for k in range(4):
                        w_chunk = wt_sbs[g16 // 4][
                            :,
                            ((g16 % 4) * 4 + k) * 128 : ((g16 % 4) * 4 + k + 1) * 128,
                        ]
                        for half in range(SUPER // 512):
                            cols = slice(half * 512, (half + 1) * 512)
                            nc.tensor.matmul(
                                ps[:, cols],
                                w_chunk,
                                z_t[:, k, cols],
                                start=(k == 0),
                                stop=(k == 3),
                            )
                    func = AF.Tanh if g16 % 4 == 3 else AF.Sigmoid
                    nc.scalar.activation(
                        acts_t[:, g16, :],
                        ps[:],
                        func,
                        bias=bias_sb[:, g16 : g16 + 1],
                    )

                # epilogue per j-chunk (contiguous [128, SUPER] slices keep
                # DVE on the 2x bf16 fast path):  c = sf + si*tg ; h = tanh(c)*so
                out_t = opool.tile([128, 4, 2, SUPER], BF16, tag="out")
                for jc in range(4):
                    sf = acts_t[:, jc * 4 + 0, :]
                    si = acts_t[:, jc * 4 + 1, :]
                    so = acts_t[:, jc * 4 + 2, :]
                    tg = acts_t[:, jc * 4 + 3, :]
                    prod = epool.tile([128, SUPER], BF16, tag="prod")
                    nc.vector.tensor_mul(prod[:], si, tg)
                    nc.vector.tensor_add(out_t[:, jc, 0, :], sf, prod[:])
                    tc_t = epool.tile([128, SUPER], BF16, tag="tc")
                    nc.scalar.activation(tc_t[:], out_t[:, jc, 0, :], AF.Tanh)
                    nc.vector.tensor_mul(out_t[:, jc, 1, :], tc_t[:], so)
                    # per-jc output DMA overlaps the remaining epilogue work
                    nc.sync.dma_start(
                        out=out[:, jc, :, b0 : b0 + SUPER], in_=out_t[:, jc, :, :]
                    )

    nc.compile()
    return nc


def pack_inputs(inputs, short_term_memory, Wf, bf, Wi, bi, Wg, bg, Wo, bo):
    x = np.asarray(inputs, np.float32)
    s = np.asarray(short_term_memory, np.float32)

    def to_pkb(a):
        # [B, H] f32 -> [core, p, k, b] bf16 with h = k*128 + p
        a16 = np.ascontiguousarray(a).astype(BF16NP)
        v = a16.reshape(N_CORES, B_CORE, 4, 128)  # [c, b, k, p]
        return np.ascontiguousarray(v.transpose(0, 3, 2, 1))  # [c, p, k, b]

    xc = to_pkb(x)
    sc = to_pkb(s)
    xs = np.ascontiguousarray(
        np.stack([xc, sc], axis=3).reshape(N_CORES * 128, 4, 2, B_CORE)
    )

    # chunk-major: column group idx16 = jc*4 + gate, inner layout [k, j]
    wt4 = np.empty((128, 16, 4, 128), BF16NP)
    for g, W in enumerate([Wf, Wi, Wo, Wg]):
        blk = (
            np.asarray(W, np.float32)
            .reshape(4, 128, 4, 128)  # [jc, j, k, p]
            .transpose(3, 0, 2, 1)  # [p, jc, k, j]
        )
        wt4[:, g::4] = blk.astype(BF16NP)
    wt = np.ascontiguousarray(wt4.reshape(128, 8192))

    bias = np.empty((128, 16), np.float32)
    for g, b in enumerate([bf, bi, bo, bg]):
        bias[:, g::4] = np.asarray(b, np.float32).reshape(4, 128).T

    return {"xs": xs, "wt": wt, "bias": bias}


def unpack_output(per_core):
    # per_core: [8, 128, 4, 2, b_core] bf16 -> [2, B, H] f32
    arr = np.transpose(per_core, (3, 0, 4, 2, 1))  # [ch, c, b, jc, p]
    return np.ascontiguousarray(arr.reshape(2, B, H).astype(np.float32))


class Runner:
    """Compiles the module once and keeps a reusable jitted executor."""

    SHARDED_INPUTS = {"xs"}

    def __init__(self, nc=None, n_cores=N_CORES):
        import jax
        from concourse import bass2jax as b2j

        self.jax = jax
        self.n_cores = n_cores
        self.nc = nc or build_module(n_cores=n_cores)
        b2j.install_neuronx_cc_hook()

        # dump the final (renamed) NEFF so neuron-profile can pair it with NTFFs
        if not getattr(b2j, "_neff_dump_patched", False):
            orig = b2j.rename_neff_tensors_and_patch_header

            def _patched(neff_path, mapping):
                data = orig(neff_path, mapping)
                with open(NEFF_DUMP, "wb") as f:
                    f.write(data)
                return data

            b2j.rename_neff_tensors_and_patch_header = _patched
            b2j._neff_dump_patched = True

        from jax.experimental.shard_map import shard_map
        from jax.sharding import Mesh, NamedSharding, PartitionSpec

        part_name = (
            self.nc.partition_id_tensor.name if self.nc.partition_id_tensor else None
        )
        in_names, out_names, out_avals = [], [], []
        self.out_shapes = {}
        for alloc in self.nc.m.functions[0].allocations:
            if not isinstance(alloc, mybir.MemoryLocationSet):
                continue
            name = alloc.memorylocations[0].name
            if alloc.kind == "ExternalInput":
                if name != part_name:
                    in_names.append(name)
            elif alloc.kind == "ExternalOutput":
                out_names.append(name)
                shape = tuple(alloc.tensor_shape)
                dt = mybir.dt.np(alloc.dtype)
                out_avals.append(jax.core.ShapedArray(shape, dt))
                self.out_shapes[name] = (shape, dt)
        self.in_names, self.out_names = in_names, out_names
        nc_ref = self.nc

        bind_names = list(in_names) + list(out_names)
        if part_name is not None:
            bind_names.append(part_name)

        def _body(*args):
            operands = list(args)
            if part_name is not None:
                operands.append(b2j.partition_id_tensor())
            outs = b2j._bass_exec_p.bind(
                *operands,
                out_avals=tuple(out_avals),
                in_names=tuple(bind_names),
                out_names=tuple(out_names),
                lowering_input_output_aliases=(),
                sim_require_finite=False,
                sim_require_nnan=False,
                nc=nc_ref,
            )
            return tuple(outs)

        devices = jax.devices()[: self.n_cores]
        mesh = Mesh(np.asarray(devices), ("core",))
        spec = PartitionSpec("core")
        n_args = len(in_names) + len(out_names)
        self.sharding = NamedSharding(mesh, spec)
        self.fn = jax.jit(
            shard_map(
                _body,
                mesh=mesh,
                in_specs=(spec,) * n_args,
                out_specs=(spec,) * len(out_names),
                check_rep=False,
            ),
            keep_unused=True,
        )
        self._dev_args = None

    def stage(self, packed):
        """Transfer inputs (sharded/replicated as needed) to devices once."""
        jax = self.jax
        nc_n = self.n_cores
        args = []
        for name in self.in_names:
            a = packed[name]
            if name in self.SHARDED_INPUTS:
                glob = a  # already per-core stacked along axis 0
            else:
                glob = np.concatenate([a] * nc_n, axis=0)  # replicate
            args.append(glob)
        for name in self.out_names:
            shape, dt = self.out_shapes[name]
            args.append(np.zeros((shape[0] * nc_n,) + shape[1:], dt))
        self._host_args = args
        self._dev_args = [jax.device_put(a, self.sharding) for a in args]

    def execute(self):
        outs = self.fn(*self._dev_args)
        self.jax.block_until_ready(outs)
        return outs

    def run(self, packed):
        self.stage(packed)
        outs = self.execute()
        res = {}
        for name, arr in zip(self.out_names, outs):
            a = np.asarray(arr)  # [n_cores*d0, ...]
            shape, _ = self.out_shapes[name]
            res[name] = a.reshape((self.n_cores, shape[0]) + tuple(shape[1:]))
        return res


_RUNNER = None


def _get_runner():
    global _RUNNER
    if _RUNNER is None:
        _RUNNER = Runner()
    return _RUNNER


def kernel(**inputs):
    r = _get_runner()
    packed = pack_inputs(**inputs)
    res = r.run(packed)
    return unpack_output(res["out"])  # [8, 128, 4, 2, 8192] -> [2, B, H]


def _sim_check():
    """CoreSim single-core validation against a numpy reference shard."""
    import jax

    from concourse.bass_interp import CoreSim

    rng = np.random.default_rng(0)
    b_core = 512  # small shard for sim speed; must be multiple of SUPER
    global B_CORE, N_SUPER, SUPER
    old = (B_CORE, N_SUPER, SUPER)
    B_CORE, SUPER = b_core, 512
    N_SUPER = 1
    try:
        nc = build_module(b_core=b_core)
        b_full = N_CORES * b_core
        x = rng.standard_normal((b_full, H)).astype(np.float32)
        s = rng.standard_normal((b_full, H)).astype(np.float32)
        sc = 1.0 / np.sqrt(H)
        ws = {k: rng.uniform(-sc, sc, (H, H)).astype(np.float32) for k in "fiog"}
        bs = {k: rng.uniform(-sc, sc, (H,)).astype(np.float32) for k in "fiog"}

        packed = pack_inputs(
            x, s, ws["f"], bs["f"], ws["i"], bs["i"], ws["g"], bs["g"],
            ws["o"], bs["o"],
        )
        sim = CoreSim(nc)
        sim.tensor("xs")[:] = packed["xs"][:128]  # core 0 shard
        sim.tensor("wt")[:] = packed["wt"]
        sim.tensor("bias")[:] = packed["bias"]
        sim.simulate(check_with_hw=False)
        got = np.asarray(sim.tensor("out"))  # [128, 4, 2, b_core] bf16
        actual = np.ascontiguousarray(
            np.transpose(got, (2, 3, 1, 0)).reshape(2, b_core, H)
        ).astype(np.float32)

        z = (x + s)[:b_core]
        sig = lambda v: 1.0 / (1.0 + np.exp(-v))
        c = sig(z @ ws["f"].T + bs["f"])
        c = c + sig(z @ ws["i"].T + bs["i"]) * np.tanh(z @ ws["g"].T + bs["g"])
        h = np.tanh(c) * sig(z @ ws["o"].T + bs["o"])
        expected = np.stack((c, h))
        rel = np.linalg.norm(actual - expected) / np.linalg.norm(expected)
        print(f"sim vs numpy rel err: {rel:.3e}")
        print(f"sim end time: {sim.time} ns (cost-model, {b_core} rows)")
        assert rel < 2e-2, "sim mismatch"
    finally:
        B_CORE, N_SUPER, SUPER = old


if __name__ == "__main__":
    if "--sim" in sys.argv:
        _sim_check()
    else:
        nc = build_module()
        print("module built + compiled OK")
